# revision 25
# baseline (speedup 1.0000x reference)
"""Trainium2 Bass kernel for nn_AttnPlus (LN -> qk proj -> per-head softmax(q k^T) @ v + A).

Sharding: 8 cores = 4 batches x 2 head-groups (8 heads each). Each core gets its
batch's x, a packed/pre-scaled slice of Wqk, its A columns; host reassembles.

Self-contained: hardcodes shapes from the problem spec.
"""

import numpy as np
import ml_dtypes

B, N, DIM, HEAD = 4, 2048, 1024, 16
HD = DIM // HEAD            # 64
HPC = HEAD // 2             # heads per core = 8
NCORES = 8
EPS = 1e-5
P = 128
NT = N // P                 # 16 row tiles
DC = DIM // P               # 8 contraction chunks
ECH = DIM // P              # 8 packed e-chunks (q/k for 8 heads = 1024 rows)
NJ = N // 512               # 4 moving-dim tiles

# Every Nth numerator chunk is computed as (gpsimd multiply + scalar-engine
# accumulate) instead of the 1x-rate DVE scalar_tensor_tensor, to balance
# engine load (DVE is otherwise the bottleneck).
GP_NUM_EVERY = 9

_CACHE = {}


def _legalize_bir(raw: bytes) -> bytes:
    """This container's walrus allows only one sync-wait command per
    instruction; Tile emits several. Split extras onto same-engine NoOp
    carriers inserted immediately before (identical semantics: waits fire
    in program order on the same engine queue before the instruction)."""
    import orjson

    m = orjson.loads(raw)
    for fn in m.get("functions", []):
        for b in fn.get("basic_blocks", fn.get("blocks", [])):
            insts = b.get("instructions", [])
            out = []
            changed = False
            for i in insts:
                si = i.get("sync_info")
                waits = si.get("on_wait") if si else None
                if waits and len(waits) > 1:
                    changed = True
                    for k, w in enumerate(waits[:-1]):
                        out.append({
                            "name": f"{i['name']}-sw{k}",
                            "opcode": "NoOp",
                            "engine": i["engine"],
                            "ins": [],
                            "outs": [],
                            "debug": i.get("debug", 0),
                            "sync_info": {"on_wait": [w], "on_update": []},
                        })
                    si["on_wait"] = [waits[-1]]
                out.append(i)
            if changed:
                b["instructions"] = out
    return orjson.dumps(m)


def _build_bass():
    import concourse.bass as bass
    import concourse.tile as tile
    from concourse import mybir
    from concourse.masks import make_identity
    from contextlib import ExitStack

    f32 = mybir.dt.float32
    bf16 = mybir.dt.bfloat16
    Alu = mybir.AluOpType
    Act = mybir.ActivationFunctionType

    nc = bass.Bass()
    x_d = nc.dram_tensor("x", [N, DIM], f32, kind="ExternalInput")
    wt_d = nc.dram_tensor("wt", [DIM, DIM], bf16, kind="ExternalInput")   # W'^T [d, e-packed]
    bias_d = nc.dram_tensor("bias", [DIM], f32, kind="ExternalInput")     # per packed e
    a_d = nc.dram_tensor("a", [N, HPC], f32, kind="ExternalInput")
    vt_d = nc.dram_tensor("vt", [HPC, N], bf16, kind="ExternalInput")
    wv_d = nc.dram_tensor("wv", [1, 1], f32, kind="ExternalInput")
    out_d = nc.dram_tensor("out", [N, HPC], f32, kind="ExternalOutput")

    with tile.TileContext(nc) as tc, ExitStack() as ctx:
        persist = ctx.enter_context(tc.tile_pool(name="persist", bufs=1))
        xp = ctx.enter_context(tc.tile_pool(name="xp", bufs=3))
        xnp = ctx.enter_context(tc.tile_pool(name="xnp", bufs=4))
        stat = ctx.enter_context(tc.tile_pool(name="stat", bufs=6))
        ep = ctx.enter_context(tc.tile_pool(name="ep", bufs=3))
        hp = ctx.enter_context(tc.tile_pool(name="hp", bufs=2))
        vstage = ctx.enter_context(tc.tile_pool(name="vstage", bufs=2))
        pp = ctx.enter_context(tc.tile_pool(name="pp", bufs=2, space="PSUM"))

        # ---------- persistent tensors ----------
        wT = persist.tile([P, DC, DIM], bf16, tag="wT", name="wT")
        xnT = persist.tile([P, DC, N], bf16, tag="xnT", name="xnT")
        qkT = persist.tile([P, ECH, N], bf16, tag="qkT", name="qkT")
        vb = persist.tile([P, HPC, N], bf16, tag="vb", name="vb")
        bias_sb = persist.tile([P, ECH], f32, tag="bias_sb", name="bias_sb")
        id_bf = persist.tile([P, P], bf16, tag="id_bf", name="id_bf")
        id_f32 = persist.tile([P, P], f32, tag="id_f32", name="id_f32")
        wv_sb = persist.tile([P, 1], f32, tag="wv_sb", name="wv_sb")
        eps_sb = persist.tile([P, 1], f32, tag="eps_sb", name="eps_sb")
        o_sb = persist.tile([NT, P * HPC], f32, tag="o_sb", name="o_sb")   # [c, nw*8+h]
        a2 = persist.tile([NT, P * HPC], f32, tag="a2", name="a2")

        # ---------- constants + input DMAs ----------
        make_identity(nc, id_bf)
        make_identity(nc, id_f32)
        nc.vector.memset(eps_sb, EPS)
        nc.gpsimd.dma_start(out=wv_sb, in_=wv_d.ap().to_broadcast([P, 1]))
        nc.sync.dma_start(out=bias_sb, in_=bias_d.rearrange("(ec p) -> p ec", p=P))
        nc.sync.dma_start(
            out=a2.rearrange("c (nw h) -> c nw h", h=HPC),
            in_=a_d.rearrange("(c nw) h -> c nw h", c=NT),
        )
        nc.sync.dma_start(out=wT, in_=wt_d.rearrange("(dc p) e -> p dc e", p=P))
        x_r = x_d.rearrange("(t p) d -> t p d", p=P)

        # ---------- LayerNorm + transpose into xnT ----------
        for t in range(NT):
            xt = xp.tile([P, DIM], f32, tag="xt", name="xt")
            nc.sync.dma_start(out=xt, in_=x_r[t])
            st = stat.tile([P, 2, 6], f32, tag="st", name="st")
            nc.vector.bn_stats(out=st[:, 0, :], in_=xt[:, 0:512])
            nc.vector.bn_stats(out=st[:, 1, :], in_=xt[:, 512:1024])
            mv = stat.tile([P, 2], f32, tag="mv", name="mv")
            nc.vector.bn_aggr(out=mv, in_=st)
            rstd = stat.tile([P, 1], f32, tag="rstd", name="rstd")
            nc.scalar.activation(out=rstd, in_=mv[:, 1:2], func=Act.Sqrt, bias=eps_sb)
            nc.vector.reciprocal(out=rstd, in_=rstd)
            xn_t = xnp.tile([P, DIM], bf16, tag="xn", name="xn")
            nc.vector.tensor_scalar(
                out=xn_t, in0=xt, scalar1=mv[:, 0:1], scalar2=rstd,
                op0=Alu.subtract, op1=Alu.mult,
            )
            tp = pp.tile([P, DIM], bf16, tag="ps", name="tp")
            for dd in range(DC):
                nc.tensor.transpose(
                    out=tp[:, dd * P : (dd + 1) * P],
                    in_=xn_t[:, dd * P : (dd + 1) * P],
                    identity=id_bf,
                )
            # one strided copy: psum [p, (dc n128)] -> xnT[:, dc, t*128:+128]
            # (on ScalarE: it is idle during the fill, DVE is not)
            nc.scalar.copy(
                out=xnT[:, :, t * P : (t + 1) * P],
                in_=tp.rearrange("p (dc n) -> p dc n", dc=DC),
            )

        # ---------- V broadcast: partition-stride-0 DMA from host vt rows
        # (no compute engine involved) ----------
        def v_prep(hi):
            nc.sync.dma_start(
                out=vb[:, hi, :],
                in_=vt_d[hi : hi + 1, :].to_broadcast([P, N]),
            )

        # ---------- projection qkT = W' @ xn^T (+bias) ----------
        def proj(ec, copy_eng=None):
            pj = pp.tile([P, N], f32, tag="ps", name="pj")
            for jt in range(NJ):
                for dd in range(DC):
                    nc.tensor.matmul(
                        out=pj[:, jt * 512 : (jt + 1) * 512],
                        lhsT=wT[:, dd, ec * P : (ec + 1) * P],
                        rhs=xnT[:, dd, jt * 512 : (jt + 1) * 512],
                        start=(dd == 0), stop=(dd == DC - 1),
                    )
            if copy_eng is None:
                nc.vector.tensor_scalar(
                    out=qkT[:, ec, :], in0=pj, scalar1=bias_sb[:, ec : ec + 1],
                    scalar2=None, op0=Alu.add,
                )
            else:
                nc.scalar.activation(
                    out=qkT[:, ec, :], in_=pj, func=Act.Identity,
                    bias=bias_sb[:, ec : ec + 1], scale=1.0,
                )

        # fill: data for pair 0 only; rest interleaves with the scores phase
        proj(0, copy_eng="act")
        proj(1, copy_eng="act")
        v_prep(0)
        v_prep(1)

        # ---------- scores + softmax + weighted sums (head pairs packed
        # into PE row-halves; interleaved matmul issue) ----------
        num_chunk_idx = 0

        def softmax_pair(pr):
            nonlocal num_chunk_idx
            dens = []
            nums = []
            for r in range(2):
                den = hp.tile([P, NT], f32, tag=f"den{r}", name=f"den{r}")
                num = hp.tile([P, NT], f32, tag=f"num{r}", name=f"num{r}")
                dens.append(den)
                nums.append(num)
            for c in range(NT):
                s_ps = [
                    pp.tile([P, N], f32, tag="ps", name=f"s_ps{r}")
                    for r in range(2)
                ]
                for jt in range(NJ):
                    for r in range(2):
                        nc.tensor.matmul(
                            out=s_ps[r][:, jt * 512 : (jt + 1) * 512],
                            lhsT=qkT[r * HD : (r + 1) * HD, 2 * pr, c * P : (c + 1) * P],
                            rhs=qkT[r * HD : (r + 1) * HD, 2 * pr + 1, jt * 512 : (jt + 1) * 512],
                            start=True, stop=True,
                        )
                for r in range(2):
                    hi = 2 * pr + r
                    e_sb = ep.tile([P, N], bf16, tag="E", name="e_sb")
                    nc.scalar.activation(
                        out=e_sb, in_=s_ps[r], func=Act.Exp,
                        accum_out=dens[r][:, c : c + 1],
                    )
                    offload = (
                        (num_chunk_idx % GP_NUM_EVERY) == (GP_NUM_EVERY - 1)
                    )
                    num_chunk_idx += 1
                    if offload:
                        # gpsimd multiplies, ScalarE accumulates
                        pmul = ep.tile([P, N], bf16, tag="pm", name="pmul")
                        nc.gpsimd.tensor_tensor(
                            out=pmul, in0=e_sb, in1=vb[:, hi, :], op=Alu.mult,
                        )
                        nc.scalar.activation(
                            out=pmul, in_=pmul, func=Act.Identity,
                            accum_out=nums[r][:, c : c + 1],
                        )
                    else:
                        nc.vector.scalar_tensor_tensor(
                            out=e_sb, in0=e_sb, scalar=1.0, in1=vb[:, hi, :],
                            op0=Alu.mult, op1=Alu.mult,
                            accum_out=nums[r][:, c : c + 1],
                        )
            for r in range(2):
                hi = 2 * pr + r
                rec = hp.tile([P, NT], f32, tag="rec", name="rec")
                nc.vector.reciprocal(out=rec, in_=dens[r])
                att = hp.tile([P, NT], f32, tag="att", name="att")
                nc.vector.scalar_tensor_tensor(
                    out=att, in0=nums[r], scalar=wv_sb, in1=rec,
                    op0=Alu.mult, op1=Alu.mult,
                )
                aps = pp.tile([NT, P], f32, tag="ps", name="aps")
                nc.tensor.transpose(out=aps, in_=att, identity=id_f32)
                nc.vector.tensor_copy(
                    out=o_sb.rearrange("c (nw h) -> c nw h", h=HPC)[:, :, hi],
                    in_=aps,
                )

        softmax_pair(0)
        proj(2)
        proj(3)
        v_prep(2)
        v_prep(3)
        softmax_pair(1)
        proj(4)
        proj(5)
        v_prep(4)
        v_prep(5)
        softmax_pair(2)
        proj(6)
        proj(7)
        v_prep(6)
        v_prep(7)
        softmax_pair(3)

        # ---------- residual + output ----------
        nc.vector.tensor_add(out=o_sb, in0=o_sb, in1=a2)
        nc.sync.dma_start(
            out=out_d.rearrange("(c nw) h -> c (nw h)", c=NT), in_=o_sb
        )

    fixed = _legalize_bir(nc.to_json_bytes())
    nc.to_json_bytes = lambda: fixed
    return nc


def _host_prep(x, A, ln_w, ln_b, Wqk, wv):
    bf = ml_dtypes.bfloat16
    scale = np.float32(DIM ** -0.5)
    W = (Wqk.astype(np.float32) * ln_w.astype(np.float32)[None, :])
    bias = Wqk.astype(np.float32) @ ln_b.astype(np.float32)
    W[:DIM] *= scale
    bias[:DIM] *= scale

    in_maps = []
    meta = []
    for core in range(NCORES):
        b, g = core // 2, core % 2
        heads = list(range(g * HPC, (g + 1) * HPC))
        e_order = []
        for p in range(HPC // 2):
            h0, h1 = heads[2 * p], heads[2 * p + 1]
            e_order += list(range(h0 * HD, (h0 + 1) * HD))
            e_order += list(range(h1 * HD, (h1 + 1) * HD))
            e_order += list(range(DIM + h0 * HD, DIM + (h0 + 1) * HD))
            e_order += list(range(DIM + h1 * HD, DIM + (h1 + 1) * HD))
        e_order = np.asarray(e_order)
        wt = np.ascontiguousarray(W[e_order].T.astype(bf))          # [d, e] bf16
        bias_c = np.ascontiguousarray(bias[e_order].astype(np.float32))
        in_maps.append({
            "x": np.ascontiguousarray(x[b].astype(np.float32)),
            "wt": wt,
            "bias": bias_c,
            "a": np.ascontiguousarray(
                A[b, :, g * HPC : (g + 1) * HPC, 0].astype(np.float32)),
            "vt": np.ascontiguousarray(
                A[b, :, g * HPC : (g + 1) * HPC, 0].T.astype(bf)),
            "wv": np.ascontiguousarray(wv.astype(np.float32)),
        })
        meta.append((b, g))
    return in_maps, meta


LAST_EXEC_NS = None


def kernel(x, A, ln_w, ln_b, Wqk, wv):
    global LAST_EXEC_NS
    import os
    from concourse.bass_utils import run_bass_kernel_spmd

    x = np.asarray(x); A = np.asarray(A)
    ln_w = np.asarray(ln_w); ln_b = np.asarray(ln_b)
    Wqk = np.asarray(Wqk); wv = np.asarray(wv)

    if "nc" not in _CACHE:
        _CACHE["nc"] = _build_bass()
    nc = _CACHE["nc"]

    in_maps, meta = _host_prep(x, A, ln_w, ln_b, Wqk, wv)
    trace = bool(int(os.environ.get("ATTN_TRACE", "0")))
    res = run_bass_kernel_spmd(
        nc, in_maps, core_ids=list(range(NCORES)), trace=trace,
    )
    LAST_EXEC_NS = res.exec_time_ns

    out = np.zeros((B, N, HEAD, 1), dtype=np.float32)
    for core, (b, g) in enumerate(meta):
        out[b, :, g * HPC : (g + 1) * HPC, 0] = res.results[core]["out"]
    return out


# revision 29
# speedup vs baseline: 1.0629x; 1.0629x over previous
"""Trainium2 Bass kernel for nn_AttnPlus (LN -> qk proj -> per-head softmax(q k^T) @ v + A).

Sharding: 8 cores = 4 batches x 2 head-groups (8 heads each). Each core gets its
batch's x, a packed/pre-scaled slice of Wqk, its A columns; host reassembles.

Self-contained: hardcodes shapes from the problem spec.
"""

import numpy as np
import ml_dtypes

B, N, DIM, HEAD = 4, 2048, 1024, 16
HD = DIM // HEAD            # 64
HPC = HEAD // 2             # heads per core = 8
NCORES = 8
EPS = 1e-5
P = 128
NT = N // P                 # 16 row tiles
DC = DIM // P               # 8 contraction chunks
ECH = DIM // P              # 8 packed e-chunks (q/k for 8 heads = 1024 rows)
NJ = N // 512               # 4 moving-dim tiles

# Every Nth numerator chunk is computed as (gpsimd multiply + scalar-engine
# accumulate) instead of the 1x-rate DVE scalar_tensor_tensor, to balance
# engine load (DVE is otherwise the bottleneck).
GP_NUM_EVERY = 16

_CACHE = {}


def _legalize_bir(raw: bytes) -> bytes:
    """This container's walrus allows only one sync-wait command per
    instruction; Tile emits several. Split extras onto same-engine NoOp
    carriers inserted immediately before (identical semantics: waits fire
    in program order on the same engine queue before the instruction)."""
    import orjson

    m = orjson.loads(raw)
    for fn in m.get("functions", []):
        for b in fn.get("basic_blocks", fn.get("blocks", [])):
            insts = b.get("instructions", [])
            out = []
            changed = False
            for i in insts:
                si = i.get("sync_info")
                waits = si.get("on_wait") if si else None
                if waits and len(waits) > 1:
                    changed = True
                    for k, w in enumerate(waits[:-1]):
                        out.append({
                            "name": f"{i['name']}-sw{k}",
                            "opcode": "NoOp",
                            "engine": i["engine"],
                            "ins": [],
                            "outs": [],
                            "debug": i.get("debug", 0),
                            "sync_info": {"on_wait": [w], "on_update": []},
                        })
                    si["on_wait"] = [waits[-1]]
                out.append(i)
            if changed:
                b["instructions"] = out
    return orjson.dumps(m)


def _build_bass():
    import concourse.bass as bass
    import concourse.tile as tile
    from concourse import mybir
    from concourse.masks import make_identity
    from contextlib import ExitStack

    f32 = mybir.dt.float32
    bf16 = mybir.dt.bfloat16
    Alu = mybir.AluOpType
    Act = mybir.ActivationFunctionType

    nc = bass.Bass()
    x_d = nc.dram_tensor("x", [N, DIM], f32, kind="ExternalInput")
    wt_d = nc.dram_tensor("wt", [DIM, DIM], bf16, kind="ExternalInput")   # W'^T [d, e-packed]
    bias_d = nc.dram_tensor("bias", [DIM], f32, kind="ExternalInput")     # per packed e
    a_d = nc.dram_tensor("a", [N, HPC], f32, kind="ExternalInput")
    vt_d = nc.dram_tensor("vt", [HPC, N], bf16, kind="ExternalInput")
    wv_d = nc.dram_tensor("wv", [1, 1], f32, kind="ExternalInput")
    out_d = nc.dram_tensor("out", [N, HPC], f32, kind="ExternalOutput")

    with tile.TileContext(nc) as tc, ExitStack() as ctx:
        persist = ctx.enter_context(tc.tile_pool(name="persist", bufs=1))
        xp = ctx.enter_context(tc.tile_pool(name="xp", bufs=3))
        xnp = ctx.enter_context(tc.tile_pool(name="xnp", bufs=4))
        stat = ctx.enter_context(tc.tile_pool(name="stat", bufs=6))
        ep = ctx.enter_context(tc.tile_pool(name="ep", bufs=3))
        hp = ctx.enter_context(tc.tile_pool(name="hp", bufs=2))
        vstage = ctx.enter_context(tc.tile_pool(name="vstage", bufs=2))
        pp = ctx.enter_context(tc.tile_pool(name="pp", bufs=2, space="PSUM"))

        # ---------- persistent tensors ----------
        wT = persist.tile([P, DC, DIM], bf16, tag="wT", name="wT")
        xnT = persist.tile([P, DC, N], bf16, tag="xnT", name="xnT")
        qkT = persist.tile([P, ECH, N], bf16, tag="qkT", name="qkT")
        vb = persist.tile([P, HPC, N], bf16, tag="vb", name="vb")
        bias_sb = persist.tile([P, ECH], f32, tag="bias_sb", name="bias_sb")
        id_bf = persist.tile([P, P], bf16, tag="id_bf", name="id_bf")
        id_f32 = persist.tile([P, P], f32, tag="id_f32", name="id_f32")
        wv_sb = persist.tile([P, 1], f32, tag="wv_sb", name="wv_sb")
        eps_sb = persist.tile([P, 1], f32, tag="eps_sb", name="eps_sb")
        o_sb = persist.tile([NT, P * HPC], f32, tag="o_sb", name="o_sb")   # [c, nw*8+h]
        a2 = persist.tile([NT, P * HPC], f32, tag="a2", name="a2")

        # ---------- constants + input DMAs ----------
        make_identity(nc, id_bf)
        make_identity(nc, id_f32)
        nc.vector.memset(eps_sb, EPS)
        nc.gpsimd.dma_start(out=wv_sb, in_=wv_d.ap().to_broadcast([P, 1]))
        nc.sync.dma_start(out=bias_sb, in_=bias_d.rearrange("(ec p) -> p ec", p=P))
        nc.sync.dma_start(
            out=a2.rearrange("c (nw h) -> c nw h", h=HPC),
            in_=a_d.rearrange("(c nw) h -> c nw h", c=NT),
        )
        nc.sync.dma_start(out=wT, in_=wt_d.rearrange("(dc p) e -> p dc e", p=P))
        x_r = x_d.rearrange("(t p) d -> t p d", p=P)

        # ---------- LayerNorm + transpose into xnT ----------
        for t in range(NT):
            xt = xp.tile([P, DIM], f32, tag="xt", name="xt")
            nc.sync.dma_start(out=xt, in_=x_r[t])
            st = stat.tile([P, 2, 6], f32, tag="st", name="st")
            nc.vector.bn_stats(out=st[:, 0, :], in_=xt[:, 0:512])
            nc.vector.bn_stats(out=st[:, 1, :], in_=xt[:, 512:1024])
            mv = stat.tile([P, 2], f32, tag="mv", name="mv")
            nc.vector.bn_aggr(out=mv, in_=st)
            rstd = stat.tile([P, 1], f32, tag="rstd", name="rstd")
            nc.scalar.activation(out=rstd, in_=mv[:, 1:2], func=Act.Sqrt, bias=eps_sb)
            nc.vector.reciprocal(out=rstd, in_=rstd)
            xn_t = xnp.tile([P, DIM], bf16, tag="xn", name="xn")
            nc.vector.tensor_scalar(
                out=xn_t, in0=xt, scalar1=mv[:, 0:1], scalar2=rstd,
                op0=Alu.subtract, op1=Alu.mult,
            )
            tp = pp.tile([P, DIM], bf16, tag="ps", name="tp")
            for dd in range(DC):
                nc.tensor.transpose(
                    out=tp[:, dd * P : (dd + 1) * P],
                    in_=xn_t[:, dd * P : (dd + 1) * P],
                    identity=id_bf,
                )
            # one strided copy: psum [p, (dc n128)] -> xnT[:, dc, t*128:+128]
            # (on ScalarE: it is idle during the fill, DVE is not)
            nc.scalar.copy(
                out=xnT[:, :, t * P : (t + 1) * P],
                in_=tp.rearrange("p (dc n) -> p dc n", dc=DC),
            )

        # ---------- V broadcast: partition-stride-0 DMA from host vt rows
        # (no compute engine involved) ----------
        def v_prep(hi):
            nc.sync.dma_start(
                out=vb[:, hi, :],
                in_=vt_d[hi : hi + 1, :].to_broadcast([P, N]),
            )

        # ---------- projection qkT = W' @ xn^T (+bias) ----------
        # Fine-grained psum groups ([128,512] per jt) so score-phase psum
        # slots are not held hostage for a whole [128,2048] accumulation.
        def proj(ec, copy_eng=None):
            for jt in range(NJ):
                pj = pp.tile([P, 512], f32, tag="ps", name="pj")
                for dd in range(DC):
                    nc.tensor.matmul(
                        out=pj,
                        lhsT=wT[:, dd, ec * P : (ec + 1) * P],
                        rhs=xnT[:, dd, jt * 512 : (jt + 1) * 512],
                        start=(dd == 0), stop=(dd == DC - 1),
                    )
                dst = qkT[:, ec, jt * 512 : (jt + 1) * 512]
                if copy_eng is None:
                    nc.vector.tensor_scalar(
                        out=dst, in0=pj, scalar1=bias_sb[:, ec : ec + 1],
                        scalar2=None, op0=Alu.add,
                    )
                else:
                    nc.scalar.activation(
                        out=dst, in_=pj, func=Act.Identity,
                        bias=bias_sb[:, ec : ec + 1], scale=1.0,
                    )

        # fill: data for pair 0 only; rest interleaves with the scores phase
        proj(0, copy_eng="act")
        proj(1, copy_eng="act")
        v_prep(0)
        v_prep(1)

        # ---------- scores + softmax + weighted sums (head pairs packed
        # into PE row-halves; interleaved matmul issue) ----------
        num_chunk_idx = 0

        def softmax_pair(pr):
            nonlocal num_chunk_idx
            dens = []
            nums = []
            for r in range(2):
                den = hp.tile([P, NT], f32, tag=f"den{r}", name=f"den{r}")
                num = hp.tile([P, NT], f32, tag=f"num{r}", name=f"num{r}")
                dens.append(den)
                nums.append(num)
            for c in range(NT):
                s_ps = [
                    pp.tile([P, N], f32, tag="ps", name=f"s_ps{r}")
                    for r in range(2)
                ]
                for jt in range(NJ):
                    for r in range(2):
                        nc.tensor.matmul(
                            out=s_ps[r][:, jt * 512 : (jt + 1) * 512],
                            lhsT=qkT[r * HD : (r + 1) * HD, 2 * pr, c * P : (c + 1) * P],
                            rhs=qkT[r * HD : (r + 1) * HD, 2 * pr + 1, jt * 512 : (jt + 1) * 512],
                            start=True, stop=True,
                        )
                for r in range(2):
                    hi = 2 * pr + r
                    e_sb = ep.tile([P, N], bf16, tag="E", name="e_sb", bufs=4)
                    nc.scalar.activation(
                        out=e_sb, in_=s_ps[r], func=Act.Exp,
                        accum_out=dens[r][:, c : c + 1],
                    )
                    offload = (
                        (num_chunk_idx % GP_NUM_EVERY) == (GP_NUM_EVERY - 1)
                    )
                    num_chunk_idx += 1
                    if offload:
                        # gpsimd multiplies, ScalarE accumulates
                        pmul = ep.tile([P, N], bf16, tag="pm", name="pmul",
                                       bufs=2)
                        nc.gpsimd.tensor_tensor(
                            out=pmul, in0=e_sb, in1=vb[:, hi, :], op=Alu.mult,
                        )
                        nc.scalar.activation(
                            out=pmul, in_=pmul, func=Act.Identity,
                            accum_out=nums[r][:, c : c + 1],
                        )
                    else:
                        scr = ep.tile([P, N], bf16, tag="scr", name="scr",
                                      bufs=3)
                        nc.vector.scalar_tensor_tensor(
                            out=scr, in0=e_sb, scalar=1.0, in1=vb[:, hi, :],
                            op0=Alu.mult, op1=Alu.mult,
                            accum_out=nums[r][:, c : c + 1],
                        )
            for r in range(2):
                hi = 2 * pr + r
                rec = hp.tile([P, NT], f32, tag="rec", name="rec")
                nc.vector.reciprocal(out=rec, in_=dens[r])
                att = hp.tile([P, NT], f32, tag="att", name="att")
                nc.vector.scalar_tensor_tensor(
                    out=att, in0=nums[r], scalar=wv_sb, in1=rec,
                    op0=Alu.mult, op1=Alu.mult,
                )
                aps = pp.tile([NT, P], f32, tag="ps", name="aps")
                nc.tensor.transpose(out=aps, in_=att, identity=id_f32)
                nc.vector.tensor_copy(
                    out=o_sb.rearrange("c (nw h) -> c nw h", h=HPC)[:, :, hi],
                    in_=aps,
                )

        softmax_pair(0)
        proj(2)
        proj(3)
        v_prep(2)
        v_prep(3)
        softmax_pair(1)
        proj(4)
        proj(5)
        v_prep(4)
        v_prep(5)
        softmax_pair(2)
        proj(6)
        proj(7)
        v_prep(6)
        v_prep(7)
        softmax_pair(3)

        # ---------- residual + output ----------
        nc.vector.tensor_add(out=o_sb, in0=o_sb, in1=a2)
        nc.sync.dma_start(
            out=out_d.rearrange("(c nw) h -> c (nw h)", c=NT), in_=o_sb
        )

    fixed = _legalize_bir(nc.to_json_bytes())
    nc.to_json_bytes = lambda: fixed
    return nc


def _host_prep(x, A, ln_w, ln_b, Wqk, wv):
    bf = ml_dtypes.bfloat16
    scale = np.float32(DIM ** -0.5)
    W = (Wqk.astype(np.float32) * ln_w.astype(np.float32)[None, :])
    bias = Wqk.astype(np.float32) @ ln_b.astype(np.float32)
    W[:DIM] *= scale
    bias[:DIM] *= scale

    in_maps = []
    meta = []
    for core in range(NCORES):
        b, g = core // 2, core % 2
        heads = list(range(g * HPC, (g + 1) * HPC))
        e_order = []
        for p in range(HPC // 2):
            h0, h1 = heads[2 * p], heads[2 * p + 1]
            e_order += list(range(h0 * HD, (h0 + 1) * HD))
            e_order += list(range(h1 * HD, (h1 + 1) * HD))
            e_order += list(range(DIM + h0 * HD, DIM + (h0 + 1) * HD))
            e_order += list(range(DIM + h1 * HD, DIM + (h1 + 1) * HD))
        e_order = np.asarray(e_order)
        wt = np.ascontiguousarray(W[e_order].T.astype(bf))          # [d, e] bf16
        bias_c = np.ascontiguousarray(bias[e_order].astype(np.float32))
        in_maps.append({
            "x": np.ascontiguousarray(x[b].astype(np.float32)),
            "wt": wt,
            "bias": bias_c,
            "a": np.ascontiguousarray(
                A[b, :, g * HPC : (g + 1) * HPC, 0].astype(np.float32)),
            "vt": np.ascontiguousarray(
                A[b, :, g * HPC : (g + 1) * HPC, 0].T.astype(bf)),
            "wv": np.ascontiguousarray(wv.astype(np.float32)),
        })
        meta.append((b, g))
    return in_maps, meta


LAST_EXEC_NS = None


def kernel(x, A, ln_w, ln_b, Wqk, wv):
    global LAST_EXEC_NS
    import os
    from concourse.bass_utils import run_bass_kernel_spmd

    x = np.asarray(x); A = np.asarray(A)
    ln_w = np.asarray(ln_w); ln_b = np.asarray(ln_b)
    Wqk = np.asarray(Wqk); wv = np.asarray(wv)

    if "nc" not in _CACHE:
        _CACHE["nc"] = _build_bass()
    nc = _CACHE["nc"]

    in_maps, meta = _host_prep(x, A, ln_w, ln_b, Wqk, wv)
    trace = bool(int(os.environ.get("ATTN_TRACE", "0")))
    res = run_bass_kernel_spmd(
        nc, in_maps, core_ids=list(range(NCORES)), trace=trace,
    )
    LAST_EXEC_NS = res.exec_time_ns

    out = np.zeros((B, N, HEAD, 1), dtype=np.float32)
    for core, (b, g) in enumerate(meta):
        out[b, :, g * HPC : (g + 1) * HPC, 0] = res.results[core]["out"]
    return out


# revision 31
# speedup vs baseline: 1.1262x; 1.0596x over previous
"""Trainium2 Bass kernel for nn_AttnPlus (LN -> qk proj -> per-head softmax(q k^T) @ v + A).

Sharding: 8 cores = 4 batches x 2 head-groups (8 heads each). Each core gets its
batch's x, a packed/pre-scaled slice of Wqk, its A columns; host reassembles.

Self-contained: hardcodes shapes from the problem spec.
"""

import numpy as np
import ml_dtypes

B, N, DIM, HEAD = 4, 2048, 1024, 16
HD = DIM // HEAD            # 64
HPC = HEAD // 2             # heads per core = 8
NCORES = 8
EPS = 1e-5
P = 128
NT = N // P                 # 16 row tiles
DC = DIM // P               # 8 contraction chunks
ECH = DIM // P              # 8 packed e-chunks (q/k for 8 heads = 1024 rows)
NJ = N // 512               # 4 moving-dim tiles

# Every Nth numerator chunk is computed as (gpsimd multiply + scalar-engine
# accumulate) instead of the 1x-rate DVE scalar_tensor_tensor, to balance
# engine load (DVE is otherwise the bottleneck).
GP_NUM_EVERY = 10 ** 9     # disabled: the offload's pipeline stalls cost more

_CACHE = {}


def _legalize_bir(raw: bytes) -> bytes:
    """This container's walrus allows only one sync-wait command per
    instruction; Tile emits several. Split extras onto same-engine NoOp
    carriers inserted immediately before (identical semantics: waits fire
    in program order on the same engine queue before the instruction)."""
    import orjson

    m = orjson.loads(raw)
    for fn in m.get("functions", []):
        for b in fn.get("basic_blocks", fn.get("blocks", [])):
            insts = b.get("instructions", [])
            out = []
            changed = False
            for i in insts:
                si = i.get("sync_info")
                waits = si.get("on_wait") if si else None
                if waits and len(waits) > 1:
                    changed = True
                    for k, w in enumerate(waits[:-1]):
                        out.append({
                            "name": f"{i['name']}-sw{k}",
                            "opcode": "NoOp",
                            "engine": i["engine"],
                            "ins": [],
                            "outs": [],
                            "debug": i.get("debug", 0),
                            "sync_info": {"on_wait": [w], "on_update": []},
                        })
                    si["on_wait"] = [waits[-1]]
                out.append(i)
            if changed:
                b["instructions"] = out
    return orjson.dumps(m)


def _build_bass():
    import concourse.bass as bass
    import concourse.tile as tile
    from concourse import mybir
    from concourse.masks import make_identity
    from contextlib import ExitStack

    f32 = mybir.dt.float32
    bf16 = mybir.dt.bfloat16
    Alu = mybir.AluOpType
    Act = mybir.ActivationFunctionType

    nc = bass.Bass()
    x_d = nc.dram_tensor("x", [N, DIM], f32, kind="ExternalInput")
    wt_d = nc.dram_tensor("wt", [DIM, DIM], bf16, kind="ExternalInput")   # W'^T [d, e-packed]
    bias_d = nc.dram_tensor("bias", [DIM], f32, kind="ExternalInput")     # per packed e
    a_d = nc.dram_tensor("a", [N, HPC], f32, kind="ExternalInput")
    vt_d = nc.dram_tensor("vt", [HPC, N], bf16, kind="ExternalInput")
    wv_d = nc.dram_tensor("wv", [1, 1], f32, kind="ExternalInput")
    out_d = nc.dram_tensor("out", [N, HPC], f32, kind="ExternalOutput")

    with tile.TileContext(nc) as tc, ExitStack() as ctx:
        persist = ctx.enter_context(tc.tile_pool(name="persist", bufs=1))
        xp = ctx.enter_context(tc.tile_pool(name="xp", bufs=3))
        xnp = ctx.enter_context(tc.tile_pool(name="xnp", bufs=4))
        stat = ctx.enter_context(tc.tile_pool(name="stat", bufs=6))
        ep = ctx.enter_context(tc.tile_pool(name="ep", bufs=3))
        hp = ctx.enter_context(tc.tile_pool(name="hp", bufs=2))
        vstage = ctx.enter_context(tc.tile_pool(name="vstage", bufs=2))
        pp = ctx.enter_context(tc.tile_pool(name="pp", bufs=2, space="PSUM"))

        # ---------- persistent tensors ----------
        wT = persist.tile([P, DC, DIM], bf16, tag="wT", name="wT")
        xnT = persist.tile([P, DC, N], bf16, tag="xnT", name="xnT")
        qkT = persist.tile([P, ECH, N], bf16, tag="qkT", name="qkT")
        vb = persist.tile([P, HPC, N], bf16, tag="vb", name="vb")
        bias_sb = persist.tile([P, ECH], f32, tag="bias_sb", name="bias_sb")
        id_bf = persist.tile([P, P], bf16, tag="id_bf", name="id_bf")
        id_f32 = persist.tile([P, P], f32, tag="id_f32", name="id_f32")
        wv_sb = persist.tile([P, 1], f32, tag="wv_sb", name="wv_sb")
        eps_sb = persist.tile([P, 1], f32, tag="eps_sb", name="eps_sb")
        o_sb = persist.tile([NT, P * HPC], f32, tag="o_sb", name="o_sb")   # [c, nw*8+h]
        a2 = persist.tile([NT, P * HPC], f32, tag="a2", name="a2")

        # ---------- constants + input DMAs ----------
        make_identity(nc, id_bf)
        make_identity(nc, id_f32)
        nc.vector.memset(eps_sb, EPS)
        nc.gpsimd.dma_start(out=wv_sb, in_=wv_d.ap().to_broadcast([P, 1]))
        nc.sync.dma_start(out=bias_sb, in_=bias_d.rearrange("(ec p) -> p ec", p=P))
        nc.sync.dma_start(
            out=a2.rearrange("c (nw h) -> c nw h", h=HPC),
            in_=a_d.rearrange("(c nw) h -> c nw h", c=NT),
        )
        nc.sync.dma_start(out=wT, in_=wt_d.rearrange("(dc p) e -> p dc e", p=P))
        x_r = x_d.rearrange("(t p) d -> t p d", p=P)

        # ---------- LayerNorm + transpose into xnT ----------
        for t in range(NT):
            xt = xp.tile([P, DIM], f32, tag="xt", name="xt")
            nc.sync.dma_start(out=xt, in_=x_r[t])
            st = stat.tile([P, 2, 6], f32, tag="st", name="st")
            nc.vector.bn_stats(out=st[:, 0, :], in_=xt[:, 0:512])
            nc.vector.bn_stats(out=st[:, 1, :], in_=xt[:, 512:1024])
            mv = stat.tile([P, 2], f32, tag="mv", name="mv")
            nc.vector.bn_aggr(out=mv, in_=st)
            rstd = stat.tile([P, 1], f32, tag="rstd", name="rstd")
            nc.scalar.activation(out=rstd, in_=mv[:, 1:2], func=Act.Sqrt, bias=eps_sb)
            nc.vector.reciprocal(out=rstd, in_=rstd)
            xn_t = xnp.tile([P, DIM], bf16, tag="xn", name="xn")
            nc.vector.tensor_scalar(
                out=xn_t, in0=xt, scalar1=mv[:, 0:1], scalar2=rstd,
                op0=Alu.subtract, op1=Alu.mult,
            )
            tp = pp.tile([P, DIM], bf16, tag="ps", name="tp")
            for dd in range(DC):
                nc.tensor.transpose(
                    out=tp[:, dd * P : (dd + 1) * P],
                    in_=xn_t[:, dd * P : (dd + 1) * P],
                    identity=id_bf,
                )
            # one strided copy: psum [p, (dc n128)] -> xnT[:, dc, t*128:+128]
            # (on ScalarE: it is idle during the fill, DVE is not)
            nc.scalar.copy(
                out=xnT[:, :, t * P : (t + 1) * P],
                in_=tp.rearrange("p (dc n) -> p dc n", dc=DC),
            )

        # ---------- V broadcast: partition-stride-0 DMA from host vt rows
        # (no compute engine involved) ----------
        def v_prep(hi):
            nc.sync.dma_start(
                out=vb[:, hi, :],
                in_=vt_d[hi : hi + 1, :].to_broadcast([P, N]),
            )

        # ---------- projection qkT = W' @ xn^T (+bias) ----------
        # Fine-grained psum groups ([128,512] per jt) so score-phase psum
        # slots are not held hostage for a whole [128,2048] accumulation.
        def proj(ec, copy_eng=None):
            for jt in range(NJ):
                pj = pp.tile([P, 512], f32, tag="ps", name="pj")
                for dd in range(DC):
                    nc.tensor.matmul(
                        out=pj,
                        lhsT=wT[:, dd, ec * P : (ec + 1) * P],
                        rhs=xnT[:, dd, jt * 512 : (jt + 1) * 512],
                        start=(dd == 0), stop=(dd == DC - 1),
                    )
                dst = qkT[:, ec, jt * 512 : (jt + 1) * 512]
                if copy_eng is None:
                    nc.vector.tensor_scalar(
                        out=dst, in0=pj, scalar1=bias_sb[:, ec : ec + 1],
                        scalar2=None, op0=Alu.add,
                    )
                else:
                    nc.scalar.activation(
                        out=dst, in_=pj, func=Act.Identity,
                        bias=bias_sb[:, ec : ec + 1], scale=1.0,
                    )

        # fill: data for pair 0 only; rest interleaves with the scores phase
        proj(0, copy_eng="act")
        proj(1, copy_eng="act")
        v_prep(0)
        v_prep(1)

        # ---------- scores + softmax + weighted sums (head pairs packed
        # into PE row-halves; interleaved matmul issue) ----------
        num_chunk_idx = 0

        def softmax_pair(pr):
            nonlocal num_chunk_idx
            dens = []
            nums = []
            for r in range(2):
                den = hp.tile([P, NT], f32, tag=f"den{r}", name=f"den{r}")
                num = hp.tile([P, NT], f32, tag=f"num{r}", name=f"num{r}")
                dens.append(den)
                nums.append(num)
            for c in range(NT):
                s_ps = [
                    pp.tile([P, N], f32, tag="ps", name=f"s_ps{r}")
                    for r in range(2)
                ]
                for jt in range(NJ):
                    for r in range(2):
                        nc.tensor.matmul(
                            out=s_ps[r][:, jt * 512 : (jt + 1) * 512],
                            lhsT=qkT[r * HD : (r + 1) * HD, 2 * pr, c * P : (c + 1) * P],
                            rhs=qkT[r * HD : (r + 1) * HD, 2 * pr + 1, jt * 512 : (jt + 1) * 512],
                            start=True, stop=True,
                        )
                for r in range(2):
                    hi = 2 * pr + r
                    e_sb = ep.tile([P, N], bf16, tag="E", name="e_sb", bufs=4)
                    nc.scalar.activation(
                        out=e_sb, in_=s_ps[r], func=Act.Exp,
                        accum_out=dens[r][:, c : c + 1],
                    )
                    offload = (
                        (num_chunk_idx % GP_NUM_EVERY) == (GP_NUM_EVERY - 1)
                    )
                    num_chunk_idx += 1
                    if offload:
                        # gpsimd multiplies, ScalarE accumulates
                        pmul = ep.tile([P, N], bf16, tag="pm", name="pmul",
                                       bufs=2)
                        nc.gpsimd.tensor_tensor(
                            out=pmul, in0=e_sb, in1=vb[:, hi, :], op=Alu.mult,
                        )
                        nc.scalar.activation(
                            out=pmul, in_=pmul, func=Act.Identity,
                            accum_out=nums[r][:, c : c + 1],
                        )
                    else:
                        scr = ep.tile([P, N], bf16, tag="scr", name="scr",
                                      bufs=3)
                        nc.vector.scalar_tensor_tensor(
                            out=scr, in0=e_sb, scalar=1.0, in1=vb[:, hi, :],
                            op0=Alu.mult, op1=Alu.mult,
                            accum_out=nums[r][:, c : c + 1],
                        )
            for r in range(2):
                hi = 2 * pr + r
                rec = hp.tile([P, NT], f32, tag="rec", name="rec")
                nc.vector.reciprocal(out=rec, in_=dens[r])
                att = hp.tile([P, NT], f32, tag="att", name="att")
                nc.vector.scalar_tensor_tensor(
                    out=att, in0=nums[r], scalar=wv_sb, in1=rec,
                    op0=Alu.mult, op1=Alu.mult,
                )
                aps = pp.tile([NT, P], f32, tag="ps", name="aps")
                nc.tensor.transpose(out=aps, in_=att, identity=id_f32)
                nc.vector.tensor_copy(
                    out=o_sb.rearrange("c (nw h) -> c nw h", h=HPC)[:, :, hi],
                    in_=aps,
                )

        softmax_pair(0)
        proj(2, copy_eng="act")
        proj(3, copy_eng="act")
        v_prep(2)
        v_prep(3)
        softmax_pair(1)
        proj(4, copy_eng="act")
        proj(5, copy_eng="act")
        v_prep(4)
        v_prep(5)
        softmax_pair(2)
        proj(6, copy_eng="act")
        proj(7, copy_eng="act")
        v_prep(6)
        v_prep(7)
        softmax_pair(3)

        # ---------- residual + output ----------
        nc.vector.tensor_add(out=o_sb, in0=o_sb, in1=a2)
        nc.sync.dma_start(
            out=out_d.rearrange("(c nw) h -> c (nw h)", c=NT), in_=o_sb
        )

    fixed = _legalize_bir(nc.to_json_bytes())
    nc.to_json_bytes = lambda: fixed
    return nc


def _host_prep(x, A, ln_w, ln_b, Wqk, wv):
    bf = ml_dtypes.bfloat16
    scale = np.float32(DIM ** -0.5)
    W = (Wqk.astype(np.float32) * ln_w.astype(np.float32)[None, :])
    bias = Wqk.astype(np.float32) @ ln_b.astype(np.float32)
    W[:DIM] *= scale
    bias[:DIM] *= scale

    in_maps = []
    meta = []
    for core in range(NCORES):
        b, g = core // 2, core % 2
        heads = list(range(g * HPC, (g + 1) * HPC))
        e_order = []
        for p in range(HPC // 2):
            h0, h1 = heads[2 * p], heads[2 * p + 1]
            e_order += list(range(h0 * HD, (h0 + 1) * HD))
            e_order += list(range(h1 * HD, (h1 + 1) * HD))
            e_order += list(range(DIM + h0 * HD, DIM + (h0 + 1) * HD))
            e_order += list(range(DIM + h1 * HD, DIM + (h1 + 1) * HD))
        e_order = np.asarray(e_order)
        wt = np.ascontiguousarray(W[e_order].T.astype(bf))          # [d, e] bf16
        bias_c = np.ascontiguousarray(bias[e_order].astype(np.float32))
        in_maps.append({
            "x": np.ascontiguousarray(x[b].astype(np.float32)),
            "wt": wt,
            "bias": bias_c,
            "a": np.ascontiguousarray(
                A[b, :, g * HPC : (g + 1) * HPC, 0].astype(np.float32)),
            "vt": np.ascontiguousarray(
                A[b, :, g * HPC : (g + 1) * HPC, 0].T.astype(bf)),
            "wv": np.ascontiguousarray(wv.astype(np.float32)),
        })
        meta.append((b, g))
    return in_maps, meta


LAST_EXEC_NS = None


def kernel(x, A, ln_w, ln_b, Wqk, wv):
    global LAST_EXEC_NS
    import os
    from concourse.bass_utils import run_bass_kernel_spmd

    x = np.asarray(x); A = np.asarray(A)
    ln_w = np.asarray(ln_w); ln_b = np.asarray(ln_b)
    Wqk = np.asarray(Wqk); wv = np.asarray(wv)

    if "nc" not in _CACHE:
        _CACHE["nc"] = _build_bass()
    nc = _CACHE["nc"]

    in_maps, meta = _host_prep(x, A, ln_w, ln_b, Wqk, wv)
    trace = bool(int(os.environ.get("ATTN_TRACE", "0")))
    res = run_bass_kernel_spmd(
        nc, in_maps, core_ids=list(range(NCORES)), trace=trace,
    )
    LAST_EXEC_NS = res.exec_time_ns

    out = np.zeros((B, N, HEAD, 1), dtype=np.float32)
    for core, (b, g) in enumerate(meta):
        out[b, :, g * HPC : (g + 1) * HPC, 0] = res.results[core]["out"]
    return out
